# revision 31
# baseline (speedup 1.0000x reference)
"""Trainium2 Bass kernel for nn_MultiHead (dense transformer layer).

Strategy: pure data-parallel over batch (B=8 -> 8 NeuronCores, no collectives).
Per core: full transformer layer on one [S=1024, D=1024] batch element.

v3 design: FULLY TRANSPOSED activations ([feature partitions, seq free]) --
zero PE transposes -- with a software-pipelined emission schedule.
  - LayerNorm stats (per-token mean / mean-square) via ones-column matmuls
    (partition-dim reduction on the PE); rows broadcast back across
    partitions with gpsimd.partition_broadcast (Pool engine).
  - Softmax denominator via a ones-column appended to V (row 64 of ctx psum).
  - Attention score matmuls for the two heads of a pair sit in disjoint
    row-groups (base partitions 0/64) and are emitted interleaved so the PE
    can run them concurrently (row tiling).
  - bf16 for all matmul operands except the LN-input tensors (CT/FFT f32r,
    via f32r-writing producers) and their stats matmuls; psum always fp32.
  - Emission interleaves attention pairs into the QKV stream (amortizes the
    ACT engine's 256 exp tiles), threads residual adds + squares per pair,
    splits the last pair around the LN1-c0 stats, and overlaps each LN
    finalize/apply chain with the next matmul block.
"""
from contextlib import ExitStack

import numpy as np

S = 1024
D = 1024
H = 16
DH = 64
DFF = 4096
P = 128
B = 8
NCORES = 8
EPS = 1e-8

_RUNNER = None


# ---------------------------------------------------------------- device kernel
def build_nc():
    import concourse.mybir as mybir
    import concourse.tile as tile
    from concourse import bacc

    f32 = mybir.dt.float32
    f32r = mybir.dt.float32r
    bf16 = mybir.dt.bfloat16
    AF = mybir.ActivationFunctionType
    ALU = mybir.AluOpType

    nc = bacc.Bacc("TRN2", target_bir_lowering=False, debug=False)

    # ---- I/O -----------------------------------------------------------------
    xT = nc.declare_dram_parameter("xT", [D, S], bf16, isOutput=False)
    wq = nc.declare_dram_parameter("wq", [8, P, 8, P], bf16, isOutput=False)
    wk = nc.declare_dram_parameter("wk", [8, P, 8, P], bf16, isOutput=False)
    wv = nc.declare_dram_parameter("wv", [P, 8, D], bf16, isOutput=False)
    wp = nc.declare_dram_parameter("wp", [P, 8, D], bf16, isOutput=False)
    wf1 = nc.declare_dram_parameter("wf1", [32, P, 8, P], bf16, isOutput=False)
    wf2 = nc.declare_dram_parameter("wf2", [8, P, 32, P], bf16, isOutput=False)
    qb = nc.declare_dram_parameter("qb", [D], f32, isOutput=False)
    kb = nc.declare_dram_parameter("kb", [D], f32, isOutput=False)
    vb = nc.declare_dram_parameter("vb", [D], bf16, isOutput=False)
    f1b = nc.declare_dram_parameter("f1b", [DFF], f32, isOutput=False)
    f2b = nc.declare_dram_parameter("f2b", [D], f32, isOutput=False)
    pb = nc.declare_dram_parameter("pb", [D], bf16, isOutput=False)
    lng = nc.declare_dram_parameter("lng", [D], f32, isOutput=False)
    lnb = nc.declare_dram_parameter("lnb", [D], f32, isOutput=False)
    fflng = nc.declare_dram_parameter("fflng", [D], f32, isOutput=False)
    fflnb = nc.declare_dram_parameter("fflnb", [D], f32, isOutput=False)
    ones1b = nc.declare_dram_parameter("ones1b", [1, P], bf16, isOutput=False)
    onescol = nc.declare_dram_parameter("onescol", [P, 1], f32r, isOutput=False)
    onespp = nc.declare_dram_parameter("onespp", [P, 1], f32, isOutput=False)
    y = nc.declare_dram_parameter("y", [S, D], f32, isOutput=True)

    def mm(out, lhsT, rhs, start, stop):
        nc.tensor.matmul(out, lhsT, rhs, start=start, stop=stop)

    with tile.TileContext(nc) as tc:
        es_top = ExitStack()

        consts = es_top.enter_context(tc.tile_pool(name="consts", bufs=1))
        mid = es_top.enter_context(tc.tile_pool(name="mid", bufs=1))
        rowp = es_top.enter_context(tc.tile_pool(name="rowp", bufs=1))
        rowbp = es_top.enter_context(tc.tile_pool(name="rowbp", bufs=2))
        scp = es_top.enter_context(tc.tile_pool(name="scp", bufs=2))
        es_qkv = ExitStack()
        xtp = es_qkv.enter_context(tc.tile_pool(name="xtp", bufs=1))
        qkvp = es_qkv.enter_context(tc.tile_pool(name="qkvp", bufs=1))

        # ---- tiles (allocation only; DMA emission is scheduled below) --------
        on1b = consts.tile([1, P], bf16, tag="on1b")
        onc = consts.tile([P, 1], f32r, tag="onc")
        onpp = consts.tile([P, 1], f32, tag="onpp")
        eps_t = consts.tile([1, 1], f32, tag="eps")
        qb_sb = consts.tile([P, 8], f32, tag="qb")
        kb_sb = consts.tile([P, 8], f32, tag="kb")
        f1b_sb = consts.tile([P, 32], f32, tag="f1b")
        f2b_sb = consts.tile([P, 8], f32, tag="f2b")
        gb1 = consts.tile([P, 8], f32, tag="gb1")
        bb1 = consts.tile([P, 8], f32, tag="bb1")
        gb2 = consts.tile([P, 8], f32, tag="gb2")
        bb2 = consts.tile([P, 8], f32, tag="bb2")
        vb_row = consts.tile([1, D], bf16, tag="vbrow")
        pb_row = consts.tile([1, D], bf16, tag="pbrow")

        CT = mid.tile([P, 8, S], f32r, tag="ctff")     # ctx+resid, later FFT
        SQ = rowp.tile([P, 8, 512], f32r, tag="sq")    # squares (per c-half)
        O1T = mid.tile([P, 8, S], bf16, tag="o1t")
        XT = xtp.tile([P, 8, S], bf16, tag="xt")
        QT = qkvp.tile([P, 8, S], bf16, tag="qt")
        KT = qkvp.tile([P, 8, S], bf16, tag="kt")
        Vp = qkvp.tile([P, 8, H * (DH + 1)], bf16, tag="vp")
        Vp5 = Vp[:].rearrange("p i (hh e) -> p i hh e", e=DH + 1)

        # critical-path DMAs first: the first stationary weight tile, then XT
        xTr = xT[:].rearrange("(ko p) s -> p ko s", p=P)

        es_ph1 = ExitStack()
        w1p = es_ph1.enter_context(tc.tile_pool(name="w1p", bufs=3))
        wj0q = w1p.tile([P, 8, P], bf16, tag="wqk", name="wj0q")
        nc.sync.dma_start(wj0q[:], wq[0])
        for k in range(8):
            nc.sync.dma_start(XT[:, k, :], xTr[:, k, :])
        wj0k = w1p.tile([P, 8, P], bf16, tag="wqk", name="wj0k")
        nc.sync.dma_start(wj0k[:], wk[0])
        wj1q = w1p.tile([P, 8, P], bf16, tag="wqk", name="wj1q")
        nc.sync.dma_start(wj1q[:], wq[1])
        nc.sync.dma_start(qb_sb[:], qb[:].rearrange("(j p) -> p j", p=P))
        nc.sync.dma_start(kb_sb[:], kb[:].rearrange("(j p) -> p j", p=P))
        wvp = es_ph1.enter_context(tc.tile_pool(name="wvp", bufs=1))
        etp = es_ph1.enter_context(tc.tile_pool(name="etp", bufs=8))
        drp = es_ph1.enter_context(tc.tile_pool(name="drp", bufs=2))
        ps_sp = es_ph1.enter_context(
            tc.tile_pool(name="ps_sp", bufs=2, space="PSUM"))
        ps_cp = es_ph1.enter_context(
            tc.tile_pool(name="ps_cp", bufs=1, space="PSUM"))
        es_qkps = ExitStack()
        ps_qk = es_qkps.enter_context(
            tc.tile_pool(name="ps_qk", bufs=1, space="PSUM"))

        def emit_qk(j, pre=None):
            """Q and K projections for feature block j. psum pair tile:
            c0 -> bank 0, c1 -> bank 1, one fused relu evac."""
            for wi, (wdram, bias_sb, out) in enumerate(
                    ((wq, qb_sb, QT), (wk, kb_sb, KT))):
                if pre is not None and pre[wi] is not None:
                    wj = pre[wi]
                else:
                    wj = w1p.tile([P, 8, P], bf16, tag="wqk")
                    nc.sync.dma_start(wj[:], wdram[j])
                pt = ps_qk.tile([P, S], f32, tag="pqk")
                for c in range(2):
                    for k in range(8):
                        mm(pt[:, c * 512:(c + 1) * 512], wj[:, k, :],
                           XT[:, k, c * 512:(c + 1) * 512],
                           start=(k == 0), stop=(k == 7))
                # relu(x+b) on DVE: keeps phase-A ACT nearly exp-only
                nc.vector.tensor_scalar(out[:, j, :], pt[:],
                                        bias_sb[:, j:j + 1], 0.0,
                                        ALU.add, ALU.max)

        def emit_v(c):
            """V projection for dout half c (heads 8c..8c+7)."""
            wvc = wvp.tile([P, 8, 512], bf16, tag="wvc")
            for k in range(8):
                nc.sync.dma_start(wvc[:, k, :], wv[:, k, c * 512:(c + 1) * 512])
            for i2 in range(4):
                pv = ps_qk.tile([P, S], f32, tag="pqk")
                for io in range(2):
                    i = 2 * i2 + io
                    hv = slice(io * 512, (io + 1) * 512)
                    for k in range(8):
                        mm(pv[:, hv], XT[:, k, i * 128:(i + 1) * 128],
                           wvc[:, k, :], start=(k == 0), stop=False)
                    mm(pv[:, hv], on1b[:], vb_row[:, c * 512:(c + 1) * 512],
                       start=False, stop=True)
                pv4 = pv[:].rearrange("p (io hh e) -> p io hh e", io=2, e=DH)
                nc.scalar.activation(
                    Vp5[:, 2 * i2:2 * i2 + 2, c * 8:(c + 1) * 8, 0:DH],
                    pv4[:], AF.Relu)

        def emit_attn(j, cset=(0, 1), cp_pool=None):
            """Attention for head pair (2j, 2j+1), transposed layout."""
            cpp = cp_pool or ps_cp
            for c in cset:
                cs = slice(c * 512, (c + 1) * 512)
                ets = {}
                # scores (row-tiled pairs into a 2-bank psum tile) + fused exp
                for t in range(8):
                    sp = ps_sp.tile([P, S], f32, tag="sp")
                    for u in range(2):
                        r0 = 64 * u
                        mm(sp[:, u * 512:(u + 1) * 512],
                           KT[r0:r0 + 64, j, t * 128:(t + 1) * 128],
                           QT[r0:r0 + 64, j, cs], start=True, stop=True)
                    et = etp.tile([P, S], bf16, tag="et")
                    nc.scalar.activation(et[:], sp[:], AF.Exp, scale=0.125)
                    ets[t] = et
                # ctx accumulation (u0 -> bank 0, u1 -> bank 1)
                cp = cpp.tile([65, S], f32, tag="cp", name=f"cp_{j}_{c}")
                for t in range(8):
                    for u in range(2):
                        mm(cp[:, u * 512:(u + 1) * 512],
                           Vp5[:, t, 2 * j + u, :],
                           ets[t][:, u * 512:(u + 1) * 512],
                           start=(t == 0), stop=(t == 7))
                # normalize by denominator row + write CT
                dr = drp.tile([1, S], f32, tag="dr")
                nc.vector.reciprocal(dr[:], cp[64:65, :])
                db = drp.tile([64, S], f32, tag="db")
                nc.gpsimd.partition_broadcast(db[:], dr[:], channels=64)
                for u in range(2):
                    r0 = 64 * u
                    nc.vector.tensor_tensor(CT[r0:r0 + 64, j, cs],
                                            cp[0:64, u * 512:(u + 1) * 512],
                                            db[:, u * 512:(u + 1) * 512],
                                            ALU.mult)

        # ------- LayerNorm building blocks (transposed layout) ----------------
        def emit_resid(dst, other, j, cs):
            nc.vector.tensor_tensor(dst[:, j, cs], dst[:, j, cs],
                                    other[:, j, cs], ALU.add)

        def emit_sq(c, src, js, eng):
            cs = slice(c * 512, (c + 1) * 512)
            for j in js:
                eng.tensor_tensor(SQ[:, j, :], src[:, j, cs], src[:, j, cs],
                                  ALU.mult)

        def emit_stats(ln_ps, c, src, nm):
            cs = slice(c * 512, (c + 1) * 512)
            psS = ln_ps.tile([1, 512], f32, tag="sums", name=f"psS_{nm}_{c}")
            psQ = ln_ps.tile([1, 512], f32, tag="sumq", name=f"psQ_{nm}_{c}")
            for j in range(8):
                mm(psS[:], onc[:], src[:, j, cs], start=(j == 0), stop=(j == 7))
                mm(psQ[:], onc[:], SQ[:, j, :], start=(j == 0), stop=(j == 7))
            return psS, psQ

        def emit_finalize(psS, psQ):
            """mean/var -> alpha (=1/std) and r2 (=mu/std), broadcast rows."""
            mu = rowp.tile([1, 512], f32, tag="mu")
            nc.scalar.activation(mu[:], psS[:], AF.Copy, scale=1.0 / D)
            ex2 = rowp.tile([1, 512], f32, tag="ex2")
            nc.scalar.activation(ex2[:], psQ[:], AF.Copy, scale=1.0 / D)
            var = rowp.tile([1, 512], f32, tag="var")
            nc.vector.tensor_tensor(var[:], mu[:], mu[:], ALU.mult)
            nc.vector.tensor_tensor(var[:], ex2[:], var[:], ALU.subtract)
            al = rowp.tile([1, 512], f32, tag="al")
            nc.scalar.activation(al[:], var[:], AF.Sqrt, bias=eps_t[:])
            nc.vector.reciprocal(al[:], al[:])
            r2 = rowp.tile([1, 512], f32, tag="r2")
            nc.vector.tensor_tensor(r2[:], mu[:], al[:], ALU.mult)
            ab = rowbp.tile([P, 512], f32, tag="ab")
            nc.gpsimd.partition_broadcast(ab[:], al[:], channels=P)
            rb = rowbp.tile([P, 512], f32, tag="rb")
            nc.gpsimd.partition_broadcast(rb[:], r2[:], channels=P)
            return ab, rb

        def emit_apply(c, src, gcol, bcol, out, ab, rb, dve_js, js=tuple(range(8))):
            """out = (src*alpha - r2)*g + b; split across DVE and Pool."""
            cs = slice(c * 512, (c + 1) * 512)
            for j in js:
                if j in dve_js:
                    sc = scp.tile([P, 512], f32, tag="scv")
                    nc.vector.tensor_tensor(sc[:], src[:, j, cs], ab[:],
                                            ALU.mult)
                    nc.vector.tensor_tensor(sc[:], sc[:], rb[:], ALU.subtract)
                    nc.vector.tensor_scalar(out[:, j, cs], sc[:],
                                            gcol[:, j:j + 1], bcol[:, j:j + 1],
                                            ALU.mult, ALU.add)
                else:
                    sc = scp.tile([P, 512], f32, tag="scp")
                    nc.gpsimd.tensor_tensor(sc[:], src[:, j, cs], ab[:],
                                            ALU.mult)
                    nc.gpsimd.tensor_tensor(sc[:], sc[:], rb[:], ALU.subtract)
                    nc.gpsimd.tensor_scalar(out[:, j, cs], sc[:],
                                            gcol[:, j:j + 1], bcol[:, j:j + 1],
                                            ALU.mult, ALU.add)

        DVE_JS = (0, 1, 2, 3, 4, 5)

        # ---- phase A: QKV + attention, interleaved ---------------------------
        emit_qk(0, pre=(wj0q, wj0k))
        # small consts stream in behind the first weight loads
        nc.sync.dma_start(onpp[:], onespp[:])
        nc.sync.dma_start(on1b[:], ones1b[:])
        nc.sync.dma_start(vb_row[:], vb[None, :])
        vp_col = Vp[:].rearrange("p i (hh e) -> p (i hh) e", e=DH + 1)[:, :, DH]
        nc.scalar.activation(vp_col, onpp[:].to_broadcast((P, 8 * H)), AF.Copy)
        emit_qk(1, pre=(wj1q, None))
        nc.sync.dma_start(onc[:], onescol[:])
        nc.vector.memset(eps_t[:], EPS)
        nc.sync.dma_start(gb1[:], lng[:].rearrange("(j p) -> p j", p=P))
        nc.sync.dma_start(bb1[:], lnb[:].rearrange("(j p) -> p j", p=P))
        nc.sync.dma_start(gb2[:], fflng[:].rearrange("(j p) -> p j", p=P))
        nc.sync.dma_start(bb2[:], fflnb[:].rearrange("(j p) -> p j", p=P))
        nc.sync.dma_start(f1b_sb[:], f1b[:].rearrange("(j p) -> p j", p=P))
        nc.sync.dma_start(f2b_sb[:], f2b[:].rearrange("(j p) -> p j", p=P))
        nc.sync.dma_start(pb_row[:], pb[None, :])
        emit_v(0)
        emit_qk(2)
        emit_attn(0)
        emit_resid(CT, XT, 0, slice(0, S))
        emit_sq(0, CT, (0,), nc.gpsimd)
        emit_qk(3)
        emit_attn(1)
        emit_resid(CT, XT, 1, slice(0, S))
        emit_sq(0, CT, (1,), nc.gpsimd)
        emit_v(1)
        emit_attn(2)
        emit_resid(CT, XT, 2, slice(0, S))
        emit_sq(0, CT, (2,), nc.gpsimd)
        emit_qk(4)
        emit_attn(3)
        emit_resid(CT, XT, 3, slice(0, S))
        emit_sq(0, CT, (3,), nc.gpsimd)
        emit_qk(5)
        emit_attn(4)
        emit_resid(CT, XT, 4, slice(0, S))
        emit_sq(0, CT, (4,), nc.gpsimd)
        emit_qk(6)
        emit_attn(5)
        emit_resid(CT, XT, 5, slice(0, S))
        emit_sq(0, CT, (5,), nc.gpsimd)
        emit_qk(7)
        # last two pairs split by seq half so the LN1-c0 chain overlaps
        # two full c1 attention chunks
        emit_attn(6, (0,))
        emit_attn(7, (0,))
        emit_resid(CT, XT, 6, slice(0, 512))
        emit_resid(CT, XT, 7, slice(0, 512))
        emit_sq(0, CT, (6, 7), nc.gpsimd)
        emit_attn(6, (1,))
        # QK/V psum no longer needed; swap those banks to the LN1-c0 stats
        es_qkps.close()
        es_lnA = ExitStack()
        ln_psA = es_lnA.enter_context(
            tc.tile_pool(name="ln_psA", bufs=1, space="PSUM"))
        psS0, psQ0 = emit_stats(ln_psA, 0, CT, "ln1")
        ab0, rb0 = emit_finalize(psS0, psQ0)
        es_lnA.close()
        # Pool half of the LN1-c0 apply runs under attn7-c1 (Pool is idle);
        # DVE half follows so attn7-c1's normalize keeps the DVE queue.
        emit_apply(0, CT, gb1, bb1, O1T, ab0, rb0, (), js=(6, 7))
        emit_attn(7, (1,))
        emit_apply(0, CT, gb1, bb1, O1T, ab0, rb0, DVE_JS, js=(0, 1, 2, 3, 4, 5))
        emit_resid(CT, XT, 6, slice(512, 1024))
        emit_resid(CT, XT, 7, slice(512, 1024))
        emit_sq(1, CT, tuple(range(8)), nc.gpsimd)

        es_ph1.close()
        es_qkv.close()   # free XT / QT / KT / Vp

        es_ph2 = ExitStack()
        ln_ps1 = es_ph2.enter_context(
            tc.tile_pool(name="ln_ps1", bufs=1, space="PSUM"))

        # ---- phase C pools (FF + LN2 + proj) ---------------------------------
        es_ph3 = ExitStack()
        ffp = es_ph3.enter_context(tc.tile_pool(name="ffp", bufs=1))
        wf1p = es_ph3.enter_context(tc.tile_pool(name="wf1p", bufs=4))
        wf2p = es_ph3.enter_context(tc.tile_pool(name="wf2p", bufs=2))
        ytp = es_ph3.enter_context(tc.tile_pool(name="ytp", bufs=2))
        ff_ps = es_ph3.enter_context(
            tc.tile_pool(name="ff_ps", bufs=2, space="PSUM"))
        pj_ps = es_ph3.enter_context(
            tc.tile_pool(name="pj_ps", bufs=2, space="PSUM"))

        H1 = ffp.tile([P, 32, 512], bf16, tag="h1")
        O2T = ffp.tile([P, 8, S], bf16, tag="o2t")
        WP = ffp.tile([P, 8, D], bf16, tag="wp")
        FFT = mid.tile([P, 8, S], f32r, tag="ctff")  # reuse CT buffer

        def emit_ff1(c):
            cs = slice(c * 512, (c + 1) * 512)
            for m in range(32):
                wm = wf1p.tile([P, 8, P], bf16, tag="wf1")
                nc.sync.dma_start(wm[:], wf1[m])
                pt = ff_ps.tile([P, 512], f32, tag="ff")
                for k in range(8):
                    mm(pt[:], wm[:, k, :], O1T[:, k, cs],
                       start=(k == 0), stop=(k == 7))
                nc.scalar.activation(H1[:, m, :], pt[:], AF.Relu,
                                     bias=f1b_sb[:, m:m + 1])

        def emit_ff2(c, pre=()):
            cs = slice(c * 512, (c + 1) * 512)
            for j in range(8):
                if j < len(pre):
                    w2j = pre[j]
                else:
                    w2j = wf2p.tile([P, 32, P], bf16, tag="w2j")
                    nc.sync.dma_start(w2j[:], wf2[j])
                pt = ff_ps.tile([P, 512], f32, tag="ff")
                for m in range(32):
                    mm(pt[:], w2j[:, m, :], H1[:, m, :],
                       start=(m == 0), stop=(m == 31))
                # fused evac: FFT = (psum + f2b) + O1T  (bias + residual)
                nc.vector.scalar_tensor_tensor(
                    FFT[:, j, cs], pt[:], f2b_sb[:, j:j + 1],
                    O1T[:, j, cs], ALU.add, ALU.add)

        def emit_proj(iset, split_last=False):
            for i in iset:
                yt = ytp.tile([P, D], f32, tag="yt")
                pp = pj_ps.tile([P, D], f32, tag="pj")
                split = split_last and i == iset[-1]
                for dh in range(2):
                    ds_ = slice(dh * 512, (dh + 1) * 512)
                    for k in range(8):
                        mm(pp[:, ds_], O2T[:, k, i * 128:(i + 1) * 128],
                           WP[:, k, ds_], start=(k == 0), stop=False)
                    mm(pp[:, ds_], on1b[:], pb_row[:, ds_],
                       start=False, stop=True)
                    if split:
                        nc.scalar.activation(yt[:, ds_], pp[:, ds_], AF.Copy)
                        nc.sync.dma_start(y[i * 128:(i + 1) * 128, ds_],
                                          yt[:, ds_])
                if not split:
                    nc.scalar.activation(yt[:], pp[:], AF.Copy)
                    nc.sync.dma_start(y[i * 128:(i + 1) * 128, :], yt[:])

        # FF1 c0 (gated only on LN1-c0 apply); LN1 c1 chain overlaps it
        emit_ff1(0)
        psS1, psQ1 = emit_stats(ln_ps1, 1, CT, "ln1")
        ab1, rb1 = emit_finalize(psS1, psQ1)
        emit_apply(1, CT, gb1, bb1, O1T, ab1, rb1, (0, 1))
        for k in range(8):
            nc.sync.dma_start(WP[:, k, :], wp[:, k, :])
        emit_ff2(0)
        emit_ff1(1)
        # LN2 c0: chain overlaps FF1 c1 matmuls (residual fused into FF2 evac)
        emit_sq(0, FFT, tuple(range(8)), nc.gpsimd)
        psS2, psQ2 = emit_stats(ln_ps1, 0, FFT, "ln2")
        ab2, rb2 = emit_finalize(psS2, psQ2)
        emit_apply(0, FFT, gb2, bb2, O2T, ab2, rb2, (0, 1))
        emit_ff2(1)
        # LN2 c1 chain overlaps proj i0-i1 (residual fused into FF2 evac)
        emit_sq(1, FFT, tuple(range(8)), nc.gpsimd)
        emit_proj((0, 1))
        psS3, psQ3 = emit_stats(ln_ps1, 1, FFT, "ln2")
        ab3, rb3 = emit_finalize(psS3, psQ3)
        emit_proj((2, 3))
        emit_apply(1, FFT, gb2, bb2, O2T, ab3, rb3, DVE_JS)
        emit_proj((4, 5, 6, 7), split_last=True)

        es_ph3.close()
        es_ph2.close()
        es_top.close()

    nc.compile()
    return nc


# ---------------------------------------------------------------- host wrapper
class _SpmdRunner:
    """Compile once, run repeatedly (mirrors bass2jax.run_bass_via_pjrt)."""

    def __init__(self, nc, n_cores):
        import jax
        from jax.sharding import Mesh, PartitionSpec
        from jax.experimental.shard_map import shard_map
        import concourse.mybir as mybir
        from concourse import bass2jax
        from concourse.bass2jax import _bass_exec_p, install_neuronx_cc_hook

        install_neuronx_cc_hook()
        self.n_cores = n_cores
        partition_name = (
            nc.partition_id_tensor.name if nc.partition_id_tensor else None
        )
        in_names, out_names, out_avals, zero_outs = [], [], [], []
        for alloc in nc.m.functions[0].allocations:
            if not isinstance(alloc, mybir.MemoryLocationSet):
                continue
            name = alloc.memorylocations[0].name
            if alloc.kind == "ExternalInput":
                if name != partition_name:
                    in_names.append(name)
            elif alloc.kind == "ExternalOutput":
                shape = tuple(alloc.tensor_shape)
                dtype = mybir.dt.np(alloc.dtype)
                out_names.append(name)
                out_avals.append(jax.core.ShapedArray(shape, dtype))
                zero_outs.append(np.zeros(shape, dtype))
        self.in_names = in_names
        self.out_names = out_names
        self.out_avals = out_avals
        self.zero_outs = zero_outs
        n_params = len(in_names)
        n_outs = len(out_avals)
        all_in_names = in_names + out_names
        if partition_name is not None:
            all_in_names.append(partition_name)
        donate = tuple(range(n_params, n_params + n_outs))

        def _body(*args):
            operands = list(args)
            if partition_name is not None:
                operands.append(bass2jax.partition_id_tensor())
            outs = _bass_exec_p.bind(
                *operands,
                out_avals=tuple(out_avals),
                in_names=tuple(all_in_names),
                out_names=tuple(out_names),
                lowering_input_output_aliases=(),
                sim_require_finite=True,
                sim_require_nnan=True,
                nc=nc,
            )
            return tuple(outs)

        import jax as _jax
        devices = _jax.devices()[:n_cores]
        assert len(devices) == n_cores
        mesh = Mesh(np.asarray(devices), ("core",))
        in_specs = (PartitionSpec("core"),) * (n_params + n_outs)
        out_specs = (PartitionSpec("core"),) * n_outs
        self.fn = _jax.jit(
            shard_map(_body, mesh=mesh, in_specs=in_specs,
                      out_specs=out_specs, check_rep=False),
            donate_argnums=donate,
            keep_unused=True,
        )

    def prep_inputs(self, in_maps):
        per_core = [[np.asarray(m[n]) for n in self.in_names] for m in in_maps]
        return [
            np.concatenate([per_core[c][i] for c in range(self.n_cores)], axis=0)
            for i in range(len(self.in_names))
        ]

    def zeros(self):
        return [
            np.zeros((self.n_cores * z.shape[0], *z.shape[1:]), z.dtype)
            for z in self.zero_outs
        ]

    def run_device(self, concat_in):
        return self.fn(*concat_in, *self.zeros())

    def split(self, out_arrs):
        return [
            {
                name: np.asarray(out_arrs[i]).reshape(
                    self.n_cores, *self.out_avals[i].shape)[c]
                for i, name in enumerate(self.out_names)
            }
            for c in range(self.n_cores)
        ]


def make_in_maps(**inputs):
    import ml_dtypes
    BF16 = np.dtype(ml_dtypes.bfloat16)
    f32 = np.float32
    q = np.ascontiguousarray(np.asarray(inputs["queries"], dtype=f32))

    def arr(name):
        return np.ascontiguousarray(np.asarray(inputs[name], dtype=f32))

    Qw, Kw, Vw = arr("Qw"), arr("Kw"), arr("Vw")
    proj_w, ff1_w, ff2_w = arr("proj_w"), arr("ff1_w"), arr("ff2_w")

    # packed weight layouts (all-contiguous device DMAs)
    def pack_lhsT(w, nj):  # [dout, din] -> [j, p(k), ko, mc]
        return np.ascontiguousarray(
            w.reshape(nj, P, 8, P).transpose(0, 3, 2, 1))

    def pack_rhs(w):  # [dout, din] -> W^T as [p(k), ko, dout]
        return np.ascontiguousarray(
            w.T.reshape(8, P, w.shape[0]).transpose(1, 0, 2))

    shared = {
        "wq": pack_lhsT(Qw, 8).astype(BF16),
        "wk": pack_lhsT(Kw, 8).astype(BF16),
        "wv": pack_rhs(Vw).astype(BF16),
        "wp": pack_rhs(proj_w).astype(BF16),
        "wf1": pack_lhsT(ff1_w, 32).astype(BF16),
        "wf2": np.ascontiguousarray(
            ff2_w.T.reshape(32, P, 8, P).transpose(2, 1, 0, 3)).astype(BF16),
        "qb": arr("Qb"), "kb": arr("Kb"), "vb": arr("Vb").astype(BF16),
        "f1b": arr("ff1_b"), "f2b": arr("ff2_b"),
        "pb": arr("proj_b").astype(BF16),
        "lng": arr("ln_g"), "lnb": arr("ln_b"),
        "fflng": arr("ffln_g"), "fflnb": arr("ffln_b"),
        "ones1b": np.ones((1, P), dtype=f32).astype(BF16),
        "onescol": np.ones((P, 1), dtype=f32),
        "onespp": np.ones((P, 1), dtype=f32),
    }
    in_maps = []
    for b in range(B):
        m = dict(shared)
        m["xT"] = np.ascontiguousarray(q[b].T).astype(BF16)
        in_maps.append(m)
    return in_maps


def get_runner():
    global _RUNNER
    if _RUNNER is None:
        nc = build_nc()
        _RUNNER = _SpmdRunner(nc, NCORES)
    return _RUNNER


def kernel(**inputs):
    runner = get_runner()
    in_maps = make_in_maps(**inputs)
    res = runner.split(runner.run_device(runner.prep_inputs(in_maps)))
    out = np.stack([res[c]["y"] for c in range(NCORES)], axis=0)
    return out.astype(np.float32)
